# revision 5
# baseline (speedup 1.0000x reference)
"""LogEntmax15 Trainium2 kernel.

Computes out = log(entmax15(x, axis=-1)) for x of shape (1, 16, 2048, 2048),
sharded row-wise across 8 NeuronCores (4096 rows each).

Algorithm (per row, z = (x - max(x))/2, entmax threshold tau solves
sum(relu(z - tau)^2) = 1):
  1. top-8 values via DVE max8 -> exact entmax threshold of the top-8
     prefix (tau8 <= tau*, a tight lower bound).
  2. Two Newton steps on f(t) = sum relu(z-t)^2 - 1 (monotone from below).
  3. One exact refit replicating the reference's mean - sqrt((1-ss)/k)
     formula on the (now exact) support set.
  4. out = 2*ln(relu(z - tau)); Ln(0) = -inf on the ACT engine matches the
     reference's log(0) = -inf positions.
"""
import sys

sys.path.insert(0, "/opt/trn_rl_repo")

import numpy as np

import concourse.bacc as bacc
import concourse.tile as tile
import concourse.mybir as mybir
from concourse.bass_utils import run_bass_kernel_spmd

f32 = mybir.dt.float32
AF = mybir.ActivationFunctionType
ALU = mybir.AluOpType

P = 128          # partitions (rows per tile)
N = 2048         # row length
ROWS = 4096      # rows per core
G = 4            # tiles per group (batched scalar chains)
NTILES = ROWS // P
NGROUPS = NTILES // G

_CACHE = {}


def build_kernel():
    nc = bacc.Bacc("TRN2", debug=False, num_devices=8)
    x = nc.dram_tensor("x", [ROWS, N], f32, kind="ExternalInput")
    out = nc.dram_tensor("out", [ROWS, N], f32, kind="ExternalOutput")

    # host-built constant tables for the segmented top-8 chain, [P, G*8]
    seg = np.tile(np.arange(8, dtype=np.float32), G)
    mask64 = np.broadcast_to((seg != 0).astype(np.float32), (P, G * 8)).copy()
    invk64 = np.broadcast_to((1.0 / (seg + 1)).astype(np.float32), (P, G * 8)).copy()
    k64 = np.broadcast_to((seg + 1).astype(np.float32), (P, G * 8)).copy()
    iota64 = np.broadcast_to(seg, (P, G * 8)).copy()

    c_mask = nc.inline_tensor(mask64, name="c_mask")
    c_invk = nc.inline_tensor(invk64, name="c_invk")
    c_k = nc.inline_tensor(k64, name="c_k")
    c_iota = nc.inline_tensor(iota64, name="c_iota")

    with tile.TileContext(nc) as tc:
        with (
            tc.tile_pool(name="consts", bufs=1) as consts,
            tc.tile_pool(name="xpool", bufs=3) as xpool,
            tc.tile_pool(name="zpool", bufs=2) as zpool,
            tc.tile_pool(name="rpool", bufs=2) as rpool,
            tc.tile_pool(name="jpool", bufs=2) as jpool,
            tc.tile_pool(name="opool", bufs=2) as opool,
            tc.tile_pool(name="spool", bufs=3) as spool,
        ):
            maskt = consts.tile([P, G * 8], f32)
            nc.sync.dma_start(out=maskt, in_=c_mask[:, :])
            invkt = consts.tile([P, G * 8], f32)
            nc.sync.dma_start(out=invkt, in_=c_invk[:, :])
            kt = consts.tile([P, G * 8], f32)
            nc.sync.dma_start(out=kt, in_=c_k[:, :])
            iotat = consts.tile([P, G * 8], f32)
            nc.sync.dma_start(out=iotat, in_=c_iota[:, :])

            for g in range(NGROUPS):
                # ---------- P0/P1: load, top8, z ----------
                t8s = spool.tile([P, G, 8], f32, tag="t8s")
                zts = []
                for j in range(G):
                    row0 = (g * G + j) * P
                    xt = xpool.tile([P, N], f32, tag="xt")
                    nc.sync.dma_start(out=xt, in_=x[row0:row0 + P, :])
                    nc.vector.max(t8s[:, j, :], xt)
                    zt = zpool.tile([P, N], f32, tag=f"z{j}")
                    # z = (x - mx) * 0.5, mx = row max = t8s[:, j, 0]
                    nc.gpsimd.tensor_scalar(out=zt, in0=xt,
                                            scalar1=t8s[:, j, 0:1], scalar2=0.5,
                                            op0=ALU.subtract, op1=ALU.mult)
                    zts.append(zt)

                # ---------- P2: batched top-8 entmax chain -> tau0 [P, G] ----------
                mxb = t8s[:, :, 0:1].broadcast_to([P, G, 8])
                z8t = spool.tile([P, G, 8], f32, tag="z8t")
                nc.vector.tensor_tensor(out=z8t, in0=t8s, in1=mxb, op=ALU.subtract)
                z8 = z8t.rearrange("p a b -> p (a b)")           # [P, 64]
                nc.vector.tensor_scalar_mul(out=z8, in0=z8, scalar1=0.5)
                z8q = spool.tile([P, G * 8], f32, tag="z8q")
                nc.vector.tensor_tensor(out=z8q, in0=z8, in1=z8, op=ALU.mult)
                c1 = spool.tile([P, G * 8], f32, tag="c1")
                # segmented cumsum: state = mask*state + value
                nc.vector.tensor_tensor_scan(out=c1, data0=maskt, data1=z8,
                                             initial=0.0, op0=ALU.mult, op1=ALU.add)
                c2 = spool.tile([P, G * 8], f32, tag="c2")
                nc.vector.tensor_tensor_scan(out=c2, data0=maskt, data1=z8q,
                                             initial=0.0, op0=ALU.mult, op1=ALU.add)
                m8 = spool.tile([P, G * 8], f32, tag="m8")
                nc.vector.tensor_tensor(out=m8, in0=c1, in1=invkt, op=ALU.mult)
                ms8 = spool.tile([P, G * 8], f32, tag="ms8")
                nc.vector.tensor_tensor(out=ms8, in0=c2, in1=invkt, op=ALU.mult)
                mm8 = spool.tile([P, G * 8], f32, tag="mm8")
                nc.vector.tensor_tensor(out=mm8, in0=m8, in1=m8, op=ALU.mult)
                nc.vector.tensor_tensor(out=ms8, in0=ms8, in1=mm8, op=ALU.subtract)
                nc.vector.tensor_tensor(out=ms8, in0=ms8, in1=kt, op=ALU.mult)  # ss
                # delta = (1 - ss) * invk ; clamp
                nc.vector.tensor_scalar(out=ms8, in0=ms8, scalar1=-1.0, scalar2=1.0,
                                        op0=ALU.mult, op1=ALU.add)
                nc.vector.tensor_tensor(out=ms8, in0=ms8, in1=invkt, op=ALU.mult)
                nc.vector.tensor_scalar_max(out=ms8, in0=ms8, scalar1=1e-20)
                # sqrt(delta) = exp(0.5*ln(delta))
                nc.scalar.activation(out=mm8, in_=ms8, func=AF.Ln)
                nc.scalar.activation(out=mm8, in_=mm8, func=AF.Exp, scale=0.5)
                # tau_j = mean_j - sqrt(delta_j); support predicate tau_j <= z8_j
                nc.vector.tensor_tensor(out=mm8, in0=m8, in1=mm8, op=ALU.subtract)
                pr8t = spool.tile([P, G, 8], f32, tag="pr8t")
                pr8 = pr8t.rearrange("p a b -> p (a b)")
                nc.vector.tensor_tensor(out=pr8, in0=mm8, in1=z8, op=ALU.is_le)
                sup = spool.tile([P, G, 1], f32, tag="sup")
                nc.vector.tensor_reduce(sup, pr8t,
                                        axis=mybir.AxisListType.X, op=ALU.add)
                sup2 = sup.rearrange("p a b -> p (a b)")
                nc.vector.tensor_scalar_add(out=sup2, in0=sup2, scalar1=-1.0)
                # one-hot select tau = tau_j at j = support-1 (per segment)
                supb = sup.broadcast_to([P, G, 8])
                iot3 = iotat.rearrange("p (a b) -> p a b", a=G)
                nc.vector.tensor_tensor(out=pr8t, in0=iot3, in1=supb, op=ALU.is_equal)
                nc.vector.tensor_tensor(out=pr8, in0=pr8, in1=mm8, op=ALU.mult)
                tau0t = spool.tile([P, G, 1], f32, tag="tau0t")
                nc.vector.tensor_reduce(tau0t, pr8t,
                                        axis=mybir.AxisListType.X, op=ALU.add)
                tau0 = tau0t.rearrange("p a b -> p (a b)")

                # ---------- Newton iterations + refit ----------
                tcur = tau0
                A1g = spool.tile([P, G], f32, tag="A1g")
                A2g = spool.tile([P, G], f32, tag="A2g")
                kkg = spool.tile([P, G], f32, tag="kkg")
                for it in range(3):
                    negt = spool.tile([P, G], f32, tag=f"negt{it}")
                    nc.scalar.mul(out=negt, in_=tcur, mul=-1.0)
                    for j in range(G):
                        r = rpool.tile([P, N], f32, tag="r")
                        nc.scalar.activation(out=r, in_=zts[j], func=AF.Relu,
                                             bias=negt[:, j:j + 1], scale=1.0,
                                             accum_out=A1g[:, j:j + 1])
                        junk = jpool.tile([P, N], mybir.dt.bfloat16, tag="junk")
                        nc.vector.affine_mul_reduce(out=junk, accum_out=A2g[:, j:j + 1],
                                                    in0=r, in1=r, scale=1.0, bias=0.0)
                        if it == 2:
                            junk2 = jpool.tile([P, N], mybir.dt.bfloat16, tag="junk")
                            nc.vector.tensor_scalar(out=junk2, in0=zts[j],
                                                    scalar1=tcur[:, j:j + 1], scalar2=0.0,
                                                    op0=ALU.is_gt, op1=ALU.add,
                                                    accum_out=kkg[:, j:j + 1])
                    if it < 2:
                        # Newton: t += (A2 - 1) * 0.5 / A1
                        am = spool.tile([P, G], f32, tag=f"am{it}")
                        nc.vector.tensor_scalar(out=am, in0=A2g, scalar1=1.0, scalar2=0.5,
                                                op0=ALU.subtract, op1=ALU.mult)
                        ia1 = spool.tile([P, G], f32, tag=f"ia1{it}")
                        nc.vector.reciprocal(out=ia1, in_=A1g)
                        nc.vector.tensor_tensor(out=am, in0=am, in1=ia1, op=ALU.mult)
                        tnew = spool.tile([P, G], f32, tag=f"t{it + 1}")
                        nc.vector.tensor_tensor(out=tnew, in0=tcur, in1=am, op=ALU.add)
                        tcur = tnew
                    else:
                        # exact refit: tau = mean - sqrt(max((1 - ss)/k, 0))
                        invk = spool.tile([P, G], f32, tag="invk")
                        nc.vector.reciprocal(out=invk, in_=kkg)
                        ktt = spool.tile([P, G], f32, tag="ktt")
                        nc.vector.tensor_tensor(out=ktt, in0=kkg, in1=tcur, op=ALU.mult)
                        S1 = spool.tile([P, G], f32, tag="S1")
                        nc.vector.tensor_tensor(out=S1, in0=A1g, in1=ktt, op=ALU.add)
                        t2x = spool.tile([P, G], f32, tag="t2x")
                        nc.vector.tensor_scalar_mul(out=t2x, in0=tcur, scalar1=2.0)
                        nc.vector.tensor_tensor(out=t2x, in0=t2x, in1=A1g, op=ALU.mult)
                        S2 = spool.tile([P, G], f32, tag="S2")
                        nc.vector.tensor_tensor(out=S2, in0=A2g, in1=t2x, op=ALU.add)
                        nc.vector.tensor_tensor(out=ktt, in0=ktt, in1=tcur, op=ALU.mult)
                        nc.vector.tensor_tensor(out=S2, in0=S2, in1=ktt, op=ALU.add)
                        mean = spool.tile([P, G], f32, tag="mean")
                        nc.vector.tensor_tensor(out=mean, in0=S1, in1=invk, op=ALU.mult)
                        msq = spool.tile([P, G], f32, tag="msq")
                        nc.vector.tensor_tensor(out=msq, in0=S2, in1=invk, op=ALU.mult)
                        mm = spool.tile([P, G], f32, tag="mm")
                        nc.vector.tensor_tensor(out=mm, in0=mean, in1=mean, op=ALU.mult)
                        nc.vector.tensor_tensor(out=msq, in0=msq, in1=mm, op=ALU.subtract)
                        nc.vector.tensor_tensor(out=msq, in0=msq, in1=kkg, op=ALU.mult)
                        nc.vector.tensor_scalar(out=msq, in0=msq, scalar1=-1.0, scalar2=1.0,
                                                op0=ALU.mult, op1=ALU.add)
                        nc.vector.tensor_tensor(out=msq, in0=msq, in1=invk, op=ALU.mult)
                        nc.vector.tensor_scalar_max(out=msq, in0=msq, scalar1=1e-20)
                        # sqrt via exp(0.5 ln) seed + one Newton-Raphson polish
                        seed = spool.tile([P, G], f32, tag="seed")
                        nc.scalar.activation(out=seed, in_=msq, func=AF.Ln)
                        nc.scalar.activation(out=seed, in_=seed, func=AF.Exp, scale=0.5)
                        iy = spool.tile([P, G], f32, tag="iy")
                        nc.vector.reciprocal(out=iy, in_=seed)
                        nc.vector.tensor_tensor(out=iy, in0=msq, in1=iy, op=ALU.mult)
                        nc.vector.tensor_tensor(out=iy, in0=iy, in1=seed, op=ALU.add)
                        nc.vector.tensor_scalar_mul(out=iy, in0=iy, scalar1=0.5)
                        tfin = spool.tile([P, G], f32, tag="tfin")
                        nc.vector.tensor_tensor(out=tfin, in0=mean, in1=iy, op=ALU.subtract)
                        tcur = tfin

                # ---------- final output ----------
                for j in range(G):
                    row0 = (g * G + j) * P
                    rf = rpool.tile([P, N], f32, tag="rf")
                    nc.gpsimd.tensor_scalar(out=rf, in0=zts[j],
                                            scalar1=tcur[:, j:j + 1], scalar2=0.0,
                                            op0=ALU.subtract, op1=ALU.max)
                    lnr = opool.tile([P, N], f32, tag="lnr")
                    nc.scalar.activation(out=lnr, in_=rf, func=AF.Ln)
                    nc.vector.tensor_scalar_mul(out=lnr, in0=lnr, scalar1=2.0)
                    nc.sync.dma_start(out=out[row0:row0 + P, :], in_=lnr)
    nc.compile()
    return nc


def kernel(x: np.ndarray) -> np.ndarray:
    x = np.ascontiguousarray(np.asarray(x, dtype=np.float32))
    orig_shape = x.shape
    xr = x.reshape(-1, N)
    n_rows = xr.shape[0]
    assert n_rows == 8 * ROWS, f"expected {8 * ROWS} rows, got {n_rows}"

    if "nc" not in _CACHE:
        _CACHE["nc"] = build_kernel()
    nc = _CACHE["nc"]

    in_maps = [{"x": np.ascontiguousarray(xr[c * ROWS:(c + 1) * ROWS])}
               for c in range(8)]
    res = run_bass_kernel_spmd(nc, in_maps, core_ids=list(range(8)))
    _CACHE["last_result"] = res
    outs = [res.results[c]["out"] for c in range(8)]
    return np.concatenate(outs, axis=0).reshape(orig_shape)


if __name__ == "__main__":
    rng = np.random.default_rng(0)
    xs = rng.standard_normal((1, 16, 2048, 2048)).astype(np.float32)
    y = kernel(xs)
    print("out", y.shape, y.dtype, "neginf frac", np.isneginf(y).mean())
